# revision 1
# baseline (speedup 1.0000x reference)
"""FLaGPE node encoder on 8 Trainium2 NeuronCores.

Sharding: data parallel over the graph axis, 2 graphs per core; the
small MLP/LayerNorm/linear parameters are replicated.

Algorithm: the reference builds dense random-walk stacks
rw = [I, P, ..., P^15] ([K,G,N,N]) but only consumes
(rw * blend).mean(-1), where blend = a + (1-2a)*[frag_i == frag_j].
With F = onehot(frag) ([N,32]) this collapses to

    feat[k,i] = (1/N) * ( a * (P^k 1)[i] + (1-2a) * (P^k F)[i, frag_i] )

so only M_k = P^k @ [F, 1] ([N,33]) is needed: 15 thin matmuls per
graph instead of dense N x N matrix powers.

Adjacency (duplicate edges counted) is built on-device as
adjT = V^T U from fp16 one-hot edge encodings on the tensor engine
(PSUM accumulates exact integer counts; fp16 holds them exactly).
deg falls out of the first iteration's "ones" column; the row
normalization 1/max(deg,1) rides the PSUM->SBUF copy (per-partition
scalar multiply on the vector engine).

Schedule: edge DMAs + both graphs' adjacency builds run first
(tensor engine back-to-back on 512-wide fp16 matmuls), then the two
graphs' 15 power-iteration steps are interleaved so each graph's
serial chain hides in the other's gaps; hx = x@Wx+bx fills leftover
tensor-engine slack.  Extraction is batched: M_k for 4 consecutive k
lands in one [128,4,33] buffer, one multiply (weights broadcast via
stride-0 AP) + one reduce per block of 4 steps.
"""

import numpy as np

import concourse.bacc as bacc
import concourse.bass as bass
import concourse.tile as tile
from concourse import mybir
from concourse.masks import make_identity
from concourse.bass_utils import run_bass_kernel_spmd

FP32, FP16, I32 = mybir.dt.float32, mybir.dt.float16, mybir.dt.int32
FP32R = mybir.dt.float32r
AF = mybir.ActivationFunctionType
OP = mybir.AluOpType

P = 128
G, N, E, K = 16, 512, 4096, 16
NF = 32                     # fragment classes
DIN, DPE, HID = 64, 28, 64
DX = 100                    # dim_emb - dim_pe
DOUT = DX + DPE             # 128
NCORES = 8
GPC = G // NCORES           # graphs per core = 2
NB = N // P                 # 4 node blocks / graph
EC = E // P                 # 32 edge chunks / graph
XB = GPC * N // P           # 8 x blocks / core
LN_EPS = 1e-5
MC = NF + 1                 # M columns: 32 one-hot + 1 ones
KB = 4                      # extraction batch (k's per M buffer)


def _bc4(ap, n):
    """[P, m] AP -> [P, n, m] with stride-0 middle dim."""
    return bass.AP(tensor=ap.tensor, offset=ap.offset,
                   ap=[ap.ap[0], [0, n], ap.ap[1]])


def _build():
    nc = bacc.Bacc()
    x_d = nc.declare_dram_parameter("x", [GPC * N, DIN], FP32, isOutput=False)
    e_d = nc.declare_dram_parameter("edges", [GPC, 2, E], I32, isOutput=False)
    f_d = nc.declare_dram_parameter("frags", [GPC, N], I32, isOutput=False)
    al_d = nc.declare_dram_parameter("alpha", [1, 1], FP32, isOutput=False)
    wx_d = nc.declare_dram_parameter("Wx", [DIN, DX], FP32, isOutput=False)
    bx_d = nc.declare_dram_parameter("bx", [1, DX], FP32, isOutput=False)
    w1_d = nc.declare_dram_parameter("W1", [K, HID], FP32, isOutput=False)
    b1_d = nc.declare_dram_parameter("b1", [HID, 1], FP32, isOutput=False)
    w2_d = nc.declare_dram_parameter("W2", [HID, HID], FP32, isOutput=False)
    b2_d = nc.declare_dram_parameter("b2", [HID, 1], FP32, isOutput=False)
    w3_d = nc.declare_dram_parameter("W3", [HID, DPE], FP32, isOutput=False)
    b3_d = nc.declare_dram_parameter("b3", [DPE, 1], FP32, isOutput=False)
    ga_d = nc.declare_dram_parameter("gamma", [1, DPE], FP32, isOutput=False)
    be_d = nc.declare_dram_parameter("beta", [1, DPE], FP32, isOutput=False)
    out_d = nc.declare_dram_parameter("out", [GPC * N, DOUT], FP32, isOutput=True)

    def bcast(h, n):
        a = h[0:1, 0:n]
        return bass.AP(tensor=a.tensor, offset=a.offset, ap=[[0, P], [1, n]])

    with tile.TileContext(nc) as tc:
        with (
            tc.tile_pool(name="consts", bufs=1) as consts,
            tc.tile_pool(name="epool", bufs=2) as epool,
            tc.tile_pool(name="ohp", bufs=6) as ohp,
            tc.tile_pool(name="adjp", bufs=2 * NB) as adjp,
            tc.tile_pool(name="mpool", bufs=2 * NB * GPC) as mpool,
            tc.tile_pool(name="fpool", bufs=2 * NB) as fpool,
            tc.tile_pool(name="spool", bufs=8) as spool,
            tc.tile_pool(name="opool", bufs=XB) as opool,
            tc.tile_pool(name="ps8", bufs=8, space="PSUM") as ps8,
        ):
            def pst(shape, name):
                return ps8.tile(shape, FP32, tag="ps", name=name)

            # ---------------- edge DMAs first ----------------
            ED = {}
            for g in range(GPC):
                src_i = epool.tile([P, EC], I32, tag="srci", name=f"srci{g}")
                nc.sync.dma_start(
                    out=src_i, in_=e_d[g, 0].rearrange("(j c) -> j c", c=EC))
                dst_i = epool.tile([P, EC], I32, tag="dsti", name=f"dsti{g}")
                nc.sync.dma_start(
                    out=dst_i, in_=e_d[g, 1].rearrange("(j c) -> j c", c=EC))
                fr_i = epool.tile([P, NB], I32, tag="fri", name=f"fri{g}")
                nc.sync.dma_start(
                    out=fr_i, in_=f_d[g].rearrange("(b p) -> p b", p=P))
                ED[g] = (src_i, dst_i, fr_i)

            # ---------------- constants ----------------
            al_sb = consts.tile([1, 1], FP32)
            nc.sync.dma_start(out=al_sb, in_=al_d[:, :])
            iota_i = consts.tile([P, N], I32)
            nc.gpsimd.iota(iota_i, pattern=[[1, N]], base=0, channel_multiplier=0)
            ident = consts.tile([P, P], FP32)
            make_identity(nc, ident)
            iota16 = consts.tile([P, N], FP16)
            nc.vector.tensor_copy(iota16, iota_i)
            ones_row = consts.tile([1, P], FP32)
            nc.vector.memset(ones_row, 1.0)
            eps_sb = consts.tile([P, 1], FP32)
            nc.vector.memset(eps_sb, LN_EPS)

            w1_sb = consts.tile([K, HID], FP32)
            nc.scalar.dma_start(out=w1_sb, in_=w1_d[:, :])
            w2_sb = consts.tile([HID, HID], FP32)
            nc.scalar.dma_start(out=w2_sb, in_=w2_d[:, :])
            w3_sb = consts.tile([HID, DPE], FP32)
            nc.scalar.dma_start(out=w3_sb, in_=w3_d[:, :])
            b1_sb = consts.tile([HID, 1], FP32)
            nc.scalar.dma_start(out=b1_sb, in_=b1_d[:, :])
            b2_sb = consts.tile([HID, 1], FP32)
            nc.scalar.dma_start(out=b2_sb, in_=b2_d[:, :])
            b3_sb = consts.tile([DPE, 1], FP32)
            nc.scalar.dma_start(out=b3_sb, in_=b3_d[:, :])
            ga_sb = consts.tile([P, DPE], FP32)
            nc.scalar.dma_start(out=ga_sb, in_=bcast(ga_d, DPE))
            be_sb = consts.tile([P, DPE], FP32)
            nc.scalar.dma_start(out=be_sb, in_=bcast(be_d, DPE))
            w1_16 = consts.tile([K, HID], FP16)
            nc.vector.tensor_copy(w1_16, w1_sb)
            w2_16 = consts.tile([HID, HID], FP16)
            nc.vector.tensor_copy(w2_16, w2_sb)
            w3_16 = consts.tile([HID, DPE], FP16)
            nc.vector.tensor_copy(w3_16, w3_sb)
            wxb_sb = consts.tile([DIN + 1, DX], FP32)
            nc.scalar.dma_start(out=wxb_sb[0:DIN, :], in_=wx_d[:, :])
            nc.scalar.dma_start(out=wxb_sb[DIN:DIN + 1, :], in_=bx_d[:, :])

            a_sb = consts.tile([1, 1], FP32)
            nc.scalar.activation(out=a_sb, in_=al_sb, func=AF.Sigmoid)

            # ---------------- emitters ----------------
            ST = {g: {} for g in range(GPC)}
            ots = [opool.tile([P, DOUT], FP32, tag="ot", name=f"ot{i}")
                   for i in range(XB)]
            xT_sb = consts.tile([DIN + 1, GPC * N], FP32)
            nc.vector.memset(xT_sb[DIN:DIN + 1, :], 1.0)

            def emit_prep(g):
                st = ST[g]
                src_i, dst_i, fr_i = ED[g]
                src_f = epool.tile([P, EC], FP32, tag="srcf", name=f"srcf{g}")
                nc.vector.tensor_copy(src_f, src_i)
                dst_f = epool.tile([P, EC], FP32, tag="dstf", name=f"dstf{g}")
                nc.vector.tensor_copy(dst_f, dst_i)
                st["src_f"], st["dst_f"] = src_f, dst_f
                st["fr_i"] = fr_i

            def emit_frag(g):
                st = ST[g]
                fr_f = epool.tile([P, NB], FP32, tag="frf", name=f"frf{g}")
                nc.vector.tensor_copy(fr_f, st["fr_i"])
                F16, Feat = [], []
                # M0 = [F, 1] stored as two [128, 2, MC] pair tiles
                Mp = [mpool.tile([P, 2, MC], FP16, tag="m0",
                                 name=f"m0_{g}_{pr}") for pr in range(2)]
                for b in range(NB):
                    f16 = fpool.tile([P, NF], FP16, tag="f16",
                                     name=f"f16_{g}_{b}")
                    nc.vector.tensor_scalar(
                        out=f16, in0=iota16[:, :NF], scalar1=fr_f[:, b:b + 1],
                        scalar2=None, op0=OP.is_equal)
                    nc.vector.tensor_copy(Mp[b // 2][:, b % 2, :NF], f16)
                    nc.vector.memset(Mp[b // 2][:, b % 2, NF:MC], 1.0)
                    ft = fpool.tile([P, K], FP32, tag="feat",
                                    name=f"ft{g}_{b}")
                    F16.append(f16)
                    Feat.append(ft)
                st["F16"], st["Feat"] = F16, Feat
                st["T"] = Mp            # current (pair of) T tiles
                st["M0"] = Mp

            def emit_adj_start(g):
                ST[g]["psa"] = [pst([P, N], f"psa{g}_{j}") for j in range(NB)]

            def emit_adj_chunk(g, c):
                st = ST[g]
                u16 = ohp.tile([P, N], FP16, tag="u16")
                nc.vector.tensor_scalar(
                    out=u16, in0=iota16, scalar1=st["src_f"][:, c:c + 1],
                    scalar2=None, op0=OP.is_equal)
                v16 = ohp.tile([P, N], FP16, tag="v16")
                nc.vector.tensor_scalar(
                    out=v16, in0=iota16, scalar1=st["dst_f"][:, c:c + 1],
                    scalar2=None, op0=OP.is_equal)
                for jb in range(NB):
                    nc.tensor.matmul(
                        st["psa"][jb], v16[:, jb * P:(jb + 1) * P], u16,
                        start=(c == 0), stop=(c == EC - 1))

            def emit_adj_copy(g):
                adjT = []
                for jb in range(NB):
                    at = adjp.tile([P, N], FP16, tag="adjT")
                    nc.scalar.copy(at, ST[g]["psa"][jb])
                    adjT.append(at)
                ST[g]["adjT"] = adjT

            def emit_c12():
                # c1 = (1-2a)/N, c2 = a/N; broadcast across partitions
                # via PE outer product with ones.
                c12 = consts.tile([1, 2], FP32)
                nc.vector.tensor_scalar(
                    out=c12[:, 0:1], in0=a_sb, scalar1=-2.0 / N,
                    scalar2=1.0 / N, op0=OP.mult, op1=OP.add)
                nc.vector.tensor_scalar(
                    out=c12[:, 1:2], in0=a_sb, scalar1=1.0 / N,
                    scalar2=None, op0=OP.mult)
                c12_ps = pst([P, 2], "c12ps")
                nc.tensor.matmul(c12_ps, ones_row, c12, start=True, stop=True)
                c12b = consts.tile([P, 2], FP32)
                nc.vector.tensor_copy(c12b, c12_ps)
                return c12b[:, 0:1], c12b[:, 1:2]

            def emit_w16(g, c1_col, c2_col):
                st = ST[g]
                Wt = []
                for b in range(NB):
                    w16 = fpool.tile([P, MC], FP16, tag="w16",
                                     name=f"w16_{g}_{b}")
                    nc.vector.tensor_scalar(
                        out=w16[:, :NF], in0=st["F16"][b], scalar1=c1_col,
                        scalar2=st["recip"][:, b:b + 1], op0=OP.mult,
                        op1=OP.mult)
                    nc.vector.tensor_tensor(
                        out=w16[:, NF:MC], in0=st["recip"][:, b:b + 1],
                        in1=c2_col, op=OP.mult)
                    Wt.append(w16)
                st["W"] = Wt

            def extract0(g):
                """Feat[b][:, 0] from M0 (no deg normalization)."""
                st = ST[g]
                for b in range(NB):
                    scr = spool.tile([P, MC], FP16, tag="scr")
                    nc.vector.scalar_tensor_tensor(
                        out=scr, in0=st["M0"][b // 2][:, b % 2, :],
                        scalar=st["deg"][:, b:b + 1], in1=st["W"][b],
                        op0=OP.mult, op1=OP.mult,
                        accum_out=st["Feat"][b][:, 0:1])

            def extract_batch(g, k0, nk):
                """Feat[b][:, k0:k0+nk] = recip * sum_c W * T_{k0..}."""
                st = ST[g]
                for b in range(NB):
                    pr, sub = b // 2, b % 2
                    tv = st["TB"][pr][:, 0:nk, sub, :]
                    w4 = _bc4(st["W"][b][:, :], nk)
                    prod = spool.tile([P, KB, MC], FP16, tag="prod")
                    nc.vector.tensor_tensor(
                        out=prod[:, 0:nk, :], in0=w4, in1=tv, op=OP.mult)
                    nc.vector.tensor_reduce(
                        out=st["Feat"][b][:, k0:k0 + nk],
                        in_=prod[:, 0:nk, :],
                        axis=mybir.AxisListType.X, op=OP.add)

            def step(g, k):
                st = ST[g]
                lhs = st["adjT"] if k == 1 else st["adjTs"]
                Tprev = st["T"]
                sl = (k - 1) % KB
                if sl == 0:
                    st["TB"] = [mpool.tile([P, KB, 2, MC], FP16, tag="t4",
                                           name=f"tb{g}_{k}_{pr}")
                                for pr in range(2)]
                tq = [pst([P, 2, MC], f"tq{g}_{k}_0"),
                      pst([P, 2, MC], f"tq{g}_{k}_1")]
                for ib in range(NB):
                    for jc in range(NB):
                        nc.tensor.matmul(
                            tq[ib // 2][:, ib % 2, :],
                            lhs[jc][:, ib * P:(ib + 1) * P],
                            Tprev[jc // 2][:, jc % 2, :],
                            start=(jc == 0), stop=(jc == NB - 1))
                if k == 1:
                    # deg from the ones column; recip; pre-scaled adjacency
                    recip = fpool.tile([P, NB], FP32, tag="recip",
                                       name=f"recip{g}")
                    deg = fpool.tile([P, NB], FP32, tag="deg",
                                     name=f"deg{g}")
                    for ib in range(NB):
                        nc.vector.tensor_scalar(
                            out=deg[:, ib:ib + 1],
                            in0=tq[ib // 2][:, ib % 2, NF:MC],
                            scalar1=1.0, scalar2=None, op0=OP.max)
                        nc.vector.reciprocal(recip[:, ib:ib + 1],
                                             deg[:, ib:ib + 1])
                    st["recip"] = recip
                    st["deg"] = deg
                    adjTs = []
                    for jc in range(NB):
                        ats = adjp.tile([P, N], FP16, tag="adjTs")
                        nc.vector.tensor_scalar(
                            out=ats, in0=st["adjT"][jc],
                            scalar1=recip[:, jc:jc + 1],
                            scalar2=None, op0=OP.mult)
                        adjTs.append(ats)
                    st["adjTs"] = adjTs
                nc.scalar.copy(st["TB"][0][:, sl, :, :], tq[0])
                nc.scalar.copy(st["TB"][1][:, sl, :, :], tq[1])
                st["T"] = [st["TB"][0][:, sl], st["TB"][1][:, sl]]
                if k % KB == 0:
                    extract_batch(g, k - KB + 1, KB)
                elif k == K - 1:
                    extract_batch(g, k - 2, 3)

            def emit_xt_dma(xb):
                xt = spool.tile([P, DIN], FP32, tag="xt", bufs=XB,
                                name=f"xt{xb}")
                nc.sync.dma_start(out=xt, in_=x_d[xb * P:(xb + 1) * P, :])
                return xt

            def emit_xt_tr(xb, xt):
                xtp = pst([DIN, P], f"xtp{xb}")
                nc.tensor.transpose(xtp, xt, ident)
                nc.vector.tensor_copy(xT_sb[0:DIN, xb * P:(xb + 1) * P], xtp)

            def emit_hx(xb):
                hxp = pst([P, DX], f"hxp{xb}")
                nc.tensor.matmul(
                    hxp, xT_sb[:, xb * P:(xb + 1) * P], wxb_sb,
                    start=True, stop=True)
                nc.vector.tensor_copy(ots[xb][:, 0:DX], hxp)

            def mlp_pieces(g):
                st = ST[g]
                featT = fpool.tile([K, N], FP16, tag="featT", name=f"fT{g}")
                hs = {}

                def p_ft(b):
                    ftp = pst([K, P], f"ftp{g}_{b}")
                    nc.tensor.transpose(ftp, st["Feat"][b], ident)
                    nc.scalar.copy(featT[:, b * P:(b + 1) * P], ftp)

                def p_h1():
                    h1p = pst([HID, N], f"h1p{g}")
                    nc.tensor.matmul(h1p, w1_16, featT, start=True, stop=True)
                    h1 = fpool.tile([HID, N], FP16, tag="h1", name=f"h1{g}")
                    nc.scalar.activation(out=h1, in_=h1p, func=AF.Relu,
                                         bias=b1_sb)
                    hs["h1"] = h1

                def p_h2():
                    h2p = pst([HID, N], f"h2p{g}")
                    nc.tensor.matmul(h2p, w2_16, hs["h1"], start=True,
                                     stop=True)
                    h2 = fpool.tile([HID, N], FP16, tag="h2", name=f"h2{g}")
                    nc.scalar.activation(out=h2, in_=h2p, func=AF.Relu,
                                         bias=b2_sb)
                    hs["h2"] = h2

                def p_h3():
                    h3p = pst([DPE, N], f"h3p{g}")
                    nc.tensor.matmul(h3p, w3_16, hs["h2"], start=True,
                                     stop=True)
                    h3 = fpool.tile([DPE, N], FP32, tag="h3", name=f"h3{g}")
                    nc.scalar.activation(out=h3, in_=h3p, func=AF.Relu,
                                         bias=b3_sb)
                    hs["h3"] = h3

                def p_ln(b):
                    hp = pst([P, DPE], f"hp{g}_{b}")
                    nc.tensor.transpose(
                        hp, hs["h3"][:, b * P:(b + 1) * P],
                        ident[0:DPE, 0:DPE])
                    stats = spool.tile([P, 6], FP32, tag="stats")
                    nc.vector.bn_stats(out=stats, in_=hp)
                    mv = spool.tile([P, 2], FP32, tag="mv")
                    nc.vector.bn_aggr(out=mv, in_=stats)
                    sd = spool.tile([P, 1], FP32, tag="sd")
                    nc.scalar.activation(
                        out=sd, in_=mv[:, 1:2], func=AF.Sqrt, bias=eps_sb)
                    rstd = spool.tile([P, 1], FP32, tag="rstd")
                    nc.vector.reciprocal(rstd, sd)
                    ot = ots[g * NB + b]
                    t0 = spool.tile([P, DPE], FP32, tag="t0")
                    nc.vector.tensor_scalar(
                        out=t0, in0=hp, scalar1=mv[:, 0:1], scalar2=rstd,
                        op0=OP.subtract, op1=OP.mult)
                    t1 = spool.tile([P, DPE], FP32, tag="t1")
                    nc.vector.tensor_tensor(
                        out=t1, in0=t0, in1=ga_sb, op=OP.mult)
                    nc.vector.tensor_tensor(
                        out=ot[:, DX:DOUT], in0=t1, in1=be_sb, op=OP.add)
                    xb = g * NB + b
                    nc.sync.dma_start(
                        out=out_d[xb * P:(xb + 1) * P, :], in_=ot)

                return ([lambda b=b: p_ft(b) for b in range(NB)]
                        + [p_h1, p_h2, p_h3]
                        + [lambda b=b: p_ln(b) for b in range(NB)])

            # ---------------- schedule ----------------
            emit_prep(0)
            emit_prep(1)
            c1_col, c2_col = emit_c12()
            emit_adj_start(0)
            emit_adj_start(1)
            for c in range(EC):
                emit_adj_chunk(0, c)
                emit_adj_chunk(1, c)
                if c == 2:
                    emit_frag(0)
                    emit_frag(1)
            emit_adj_copy(0)
            emit_adj_copy(1)
            xts = [emit_xt_dma(xb) for xb in range(XB)]
            fillers = ([(lambda xb=xb: emit_xt_tr(xb, xts[xb]))
                        for xb in range(XB)]
                       + [(lambda xb=xb: emit_hx(xb)) for xb in range(XB)])
            fi = 0
            for k in range(1, K):
                step(0, k)
                step(1, k)
                if k == 1:
                    emit_w16(0, c1_col, c2_col)
                    emit_w16(1, c1_col, c2_col)
                    extract0(0)
                    extract0(1)
                if fi < len(fillers):
                    fillers[fi]()
                    fi += 1
            while fi < len(fillers):
                fillers[fi]()
                fi += 1
            for p0, p1 in zip(mlp_pieces(0), mlp_pieces(1)):
                p0()
                p1()

    nc.finalize()
    return nc


_CACHE = {}


def _get_nc():
    if "nc" not in _CACHE:
        _CACHE["nc"] = _build()
    return _CACHE["nc"]


def _shard_inputs(inputs):
    x = np.ascontiguousarray(np.asarray(inputs["x"], dtype=np.float32))
    e = np.ascontiguousarray(np.asarray(inputs["edge_index"], dtype=np.int32))
    fr = np.ascontiguousarray(np.asarray(inputs["fragment_ids"], dtype=np.int32))
    al = np.asarray(inputs["alpha"], dtype=np.float32).reshape(1, 1)
    com = {
        "alpha": al,
        "Wx": np.ascontiguousarray(np.asarray(inputs["Wx"], np.float32)),
        "bx": np.asarray(inputs["bx"], np.float32).reshape(1, DX),
        "W1": np.ascontiguousarray(np.asarray(inputs["W1"], np.float32)),
        "b1": np.asarray(inputs["b1"], np.float32).reshape(HID, 1),
        "W2": np.ascontiguousarray(np.asarray(inputs["W2"], np.float32)),
        "b2": np.asarray(inputs["b2"], np.float32).reshape(HID, 1),
        "W3": np.ascontiguousarray(np.asarray(inputs["W3"], np.float32)),
        "b3": np.asarray(inputs["b3"], np.float32).reshape(DPE, 1),
        "gamma": np.asarray(inputs["gamma"], np.float32).reshape(1, DPE),
        "beta": np.asarray(inputs["beta"], np.float32).reshape(1, DPE),
    }
    in_maps = []
    for c in range(NCORES):
        g0 = c * GPC
        in_maps.append(dict(
            com,
            x=x[g0 * N:(g0 + GPC) * N],
            edges=e[g0:g0 + GPC],
            frags=fr[g0:g0 + GPC],
        ))
    return in_maps


def _run(inputs, trace=False):
    nc = _get_nc()
    in_maps = _shard_inputs(inputs)
    res = run_bass_kernel_spmd(nc, in_maps, list(range(NCORES)), trace=trace)
    out = np.concatenate([res.results[c]["out"] for c in range(NCORES)], axis=0)
    return out, res


def kernel(**inputs):
    out, _ = _run(inputs, trace=False)
    return out



# revision 7
# speedup vs baseline: 1.3599x; 1.3599x over previous
"""FLaGPE node encoder on 8 Trainium2 NeuronCores.

Sharding: data parallel over the graph axis, 2 graphs per core; the
small MLP/LayerNorm/linear parameters are replicated.

Algorithm: the reference builds dense random-walk stacks
rw = [I, P, ..., P^15] ([K,G,N,N]) but only consumes
(rw * blend).mean(-1), where blend = a + (1-2a)*[frag_i == frag_j].
With F = onehot(frag) ([N,32]) this collapses to

    feat[k,i] = (1/N) * ( a * (P^k 1)[i] + (1-2a) * (P^k F)[i, frag_i] )

so only M_k = P^k @ [F, 1] ([N,33]) is needed: 15 thin matmuls per
graph instead of dense N x N matrix powers.

Edge ingest (CSR-style host layout): the COO edge list is re-laid-out
on the host into 16 (dst-block, src-block) buckets of 128x128 blocks,
padded with sentinel entries to 3 chunks of 128 edges (the reference
is invariant to edge order; this is the standard bucketed-CSR
conversion GNN frameworks do on ingest).  The device then builds the
dense adjacency with narrow [128,128] one-hot compares and one
128-wide matmul per chunk - ~8x less tensor-engine streaming and
~2.7x less one-hot vector work than the unbucketed V^T U build.
Sentinel entries use local index 255, whose one-hot row is zero, so
they contribute nothing.

deg falls out of the first iteration's "ones" column; the row
normalization 1/max(deg,1) is folded into the extraction weights.
"""

import numpy as np

import concourse.bacc as bacc
import concourse.bass as bass
import concourse.tile as tile
from concourse import mybir
from concourse.masks import make_identity
from concourse.bass_utils import run_bass_kernel_spmd

FP32, FP16, I32 = mybir.dt.float32, mybir.dt.float16, mybir.dt.int32
AF = mybir.ActivationFunctionType
OP = mybir.AluOpType

P = 128
G, N, E, K = 16, 512, 4096, 16
NF = 32                     # fragment classes
DIN, DPE, HID = 64, 28, 64
DX = 100                    # dim_emb - dim_pe
DOUT = DX + DPE             # 128
NCORES = 8
GPC = G // NCORES           # graphs per core = 2
NB = N // P                 # 4 node blocks / graph
XB = GPC * N // P           # 8 x blocks / core
LN_EPS = 1e-5
MC = NF + 1                 # M columns: 32 one-hot + 1 ones
KB = 4                      # extraction batch (k's per M buffer)
NBK = 16                    # buckets / graph


def _build(bch):
    nch = NBK * bch         # chunk-columns per graph
    nc = bacc.Bacc()
    x_d = nc.declare_dram_parameter("x", [GPC * N, DIN], FP32, isOutput=False)
    sl_d = nc.declare_dram_parameter("sl", [GPC, P, nch], FP16, isOutput=False)
    dl_d = nc.declare_dram_parameter("dl", [GPC, P, nch], FP16, isOutput=False)
    f_d = nc.declare_dram_parameter("frags", [GPC, N], I32, isOutput=False)
    al_d = nc.declare_dram_parameter("alpha", [1, 1], FP32, isOutput=False)
    wx_d = nc.declare_dram_parameter("Wx", [DIN, DX], FP32, isOutput=False)
    bx_d = nc.declare_dram_parameter("bx", [1, DX], FP32, isOutput=False)
    w1_d = nc.declare_dram_parameter("W1", [K, HID], FP32, isOutput=False)
    b1_d = nc.declare_dram_parameter("b1", [HID, 1], FP32, isOutput=False)
    w2_d = nc.declare_dram_parameter("W2", [HID, HID], FP32, isOutput=False)
    b2_d = nc.declare_dram_parameter("b2", [HID, 1], FP32, isOutput=False)
    w3_d = nc.declare_dram_parameter("W3", [HID, DPE], FP32, isOutput=False)
    b3_d = nc.declare_dram_parameter("b3", [DPE, 1], FP32, isOutput=False)
    ga_d = nc.declare_dram_parameter("gamma", [1, DPE], FP32, isOutput=False)
    be_d = nc.declare_dram_parameter("beta", [1, DPE], FP32, isOutput=False)
    out_d = nc.declare_dram_parameter("out", [GPC * N, DOUT], FP32, isOutput=True)

    def bcast(h, n):
        a = h[0:1, 0:n]
        return bass.AP(tensor=a.tensor, offset=a.offset, ap=[[0, P], [1, n]])

    with tile.TileContext(nc) as tc:
        with (
            tc.tile_pool(name="consts", bufs=1) as consts,
            tc.tile_pool(name="epool", bufs=2) as epool,
            tc.tile_pool(name="ohp", bufs=6) as ohp,
            tc.tile_pool(name="adjp", bufs=2 * NB) as adjp,
            tc.tile_pool(name="mpool", bufs=2 * NB * GPC) as mpool,
            tc.tile_pool(name="fpool", bufs=2 * NB) as fpool,
            tc.tile_pool(name="spool", bufs=8) as spool,
            tc.tile_pool(name="opool", bufs=XB) as opool,
            tc.tile_pool(name="ps8", bufs=8, space="PSUM") as ps8,
        ):
            def pst(shape, name):
                return ps8.tile(shape, FP32, tag="ps", name=name)

            # ---------------- edge + frag DMAs first ----------------
            ED = {}
            for g in range(GPC):
                sl = epool.tile([P, nch], FP16, tag="sl", name=f"sl{g}")
                nc.sync.dma_start(out=sl, in_=sl_d[g])
                dl = epool.tile([P, nch], FP16, tag="dl", name=f"dl{g}")
                nc.sync.dma_start(out=dl, in_=dl_d[g])
                fr_i = epool.tile([P, NB], I32, tag="fri", name=f"fri{g}")
                nc.sync.dma_start(
                    out=fr_i, in_=f_d[g].rearrange("(b p) -> p b", p=P))
                ED[g] = (sl, dl, fr_i)

            # ---------------- constants ----------------
            al_sb = consts.tile([1, 1], FP32)
            nc.sync.dma_start(out=al_sb, in_=al_d[:, :])
            iota_i = consts.tile([P, N], I32)
            nc.gpsimd.iota(iota_i, pattern=[[1, N]], base=0, channel_multiplier=0)
            ident = consts.tile([P, P], FP32)
            make_identity(nc, ident)
            iota16 = consts.tile([P, N], FP16)
            nc.vector.tensor_copy(iota16, iota_i)
            ones_row = consts.tile([1, P], FP32)
            nc.vector.memset(ones_row, 1.0)
            eps_sb = consts.tile([P, 1], FP32)
            nc.vector.memset(eps_sb, LN_EPS)

            w1_sb = consts.tile([K, HID], FP32)
            nc.scalar.dma_start(out=w1_sb, in_=w1_d[:, :])
            w2_sb = consts.tile([HID, HID], FP32)
            nc.scalar.dma_start(out=w2_sb, in_=w2_d[:, :])
            w3_sb = consts.tile([HID, DPE], FP32)
            nc.scalar.dma_start(out=w3_sb, in_=w3_d[:, :])
            b1_sb = consts.tile([HID, 1], FP32)
            nc.scalar.dma_start(out=b1_sb, in_=b1_d[:, :])
            b2_sb = consts.tile([HID, 1], FP32)
            nc.scalar.dma_start(out=b2_sb, in_=b2_d[:, :])
            b3_sb = consts.tile([DPE, 1], FP32)
            nc.scalar.dma_start(out=b3_sb, in_=b3_d[:, :])
            ga_sb = consts.tile([P, DPE], FP32)
            nc.scalar.dma_start(out=ga_sb, in_=bcast(ga_d, DPE))
            be_sb = consts.tile([P, DPE], FP32)
            nc.scalar.dma_start(out=be_sb, in_=bcast(be_d, DPE))
            w1_16 = consts.tile([K, HID], FP16)
            nc.vector.tensor_copy(w1_16, w1_sb)
            w2_16 = consts.tile([HID, HID], FP16)
            nc.vector.tensor_copy(w2_16, w2_sb)
            w3_16 = consts.tile([HID, DPE], FP16)
            nc.vector.tensor_copy(w3_16, w3_sb)
            wxb_sb = consts.tile([DIN + 1, DX], FP32)
            nc.scalar.dma_start(out=wxb_sb[0:DIN, :], in_=wx_d[:, :])
            nc.scalar.dma_start(out=wxb_sb[DIN:DIN + 1, :], in_=bx_d[:, :])

            a_sb = consts.tile([1, 1], FP32)
            nc.scalar.activation(out=a_sb, in_=al_sb, func=AF.Sigmoid)

            # ---------------- emitters ----------------
            ST = {g: {} for g in range(GPC)}
            ots = [opool.tile([P, DOUT], FP32, tag="ot", name=f"ot{i}")
                   for i in range(XB)]
            xT_sb = consts.tile([DIN + 1, GPC * N], FP32)
            nc.vector.memset(xT_sb[DIN:DIN + 1, :], 1.0)

            def emit_frag(g):
                st = ST[g]
                fr_f = epool.tile([P, NB], FP32, tag="frf", name=f"frf{g}")
                nc.vector.tensor_copy(fr_f, ED[g][2])
                F16, Feat = [], []
                Mp = [mpool.tile([P, 2, MC], FP16, tag="m0",
                                 name=f"m0_{g}_{pr}") for pr in range(2)]
                for b in range(NB):
                    f16 = fpool.tile([P, NF], FP16, tag="f16",
                                     name=f"f16_{g}_{b}")
                    nc.vector.tensor_scalar(
                        out=f16, in0=iota16[:, :NF], scalar1=fr_f[:, b:b + 1],
                        scalar2=None, op0=OP.is_equal)
                    nc.vector.tensor_copy(Mp[b // 2][:, b % 2, :NF], f16)
                    nc.vector.memset(Mp[b // 2][:, b % 2, NF:MC], 1.0)
                    ft = fpool.tile([P, K], FP32, tag="feat",
                                    name=f"ft{g}_{b}")
                    F16.append(f16)
                    Feat.append(ft)
                st["F16"], st["Feat"] = F16, Feat
                st["T"] = Mp
                st["M0"] = Mp

            def emit_adj_start(g):
                ST[g]["psa"] = [pst([P, N], f"psa{g}_{j}") for j in range(NB)]

            def emit_adj_bucket(g, b):
                """One (jb, ib) bucket: batched one-hots + bch matmuls."""
                st = ST[g]
                sl, dl, _ = ED[g]
                jb, ib = b // 4, b % 4
                ia = iota16[:, :]
                i_b = bass.AP(tensor=ia.tensor, offset=ia.offset,
                              ap=[ia.ap[0], [0, bch], [1, P]])
                u16 = ohp.tile([P, bch, P], FP16, tag="u16")
                s_src = sl[:, b * bch:(b + 1) * bch]
                s_b = bass.AP(tensor=s_src.tensor, offset=s_src.offset,
                              ap=[s_src.ap[0], [1, bch], [0, P]])
                nc.vector.tensor_tensor(out=u16, in0=i_b, in1=s_b,
                                        op=OP.is_equal)
                v16 = ohp.tile([P, bch, P], FP16, tag="v16")
                d_src = dl[:, b * bch:(b + 1) * bch]
                d_b = bass.AP(tensor=d_src.tensor, offset=d_src.offset,
                              ap=[d_src.ap[0], [1, bch], [0, P]])
                nc.vector.tensor_tensor(out=v16, in0=i_b, in1=d_b,
                                        op=OP.is_equal)
                for c in range(bch):
                    nc.tensor.matmul(
                        st["psa"][jb][:, ib * P:(ib + 1) * P],
                        v16[:, c, :], u16[:, c, :],
                        start=(c == 0), stop=(c == bch - 1))

            def emit_adj_copy(g, jb):
                st = ST[g]
                if "adjT" not in st:
                    st["adjT"] = [None] * NB
                at = adjp.tile([P, N], FP16, tag="adjT")
                nc.scalar.copy(at, st["psa"][jb])
                st["adjT"][jb] = at

            def emit_c12():
                c12 = consts.tile([1, 2], FP32)
                nc.vector.tensor_scalar(
                    out=c12[:, 0:1], in0=a_sb, scalar1=-2.0 / N,
                    scalar2=1.0 / N, op0=OP.mult, op1=OP.add)
                nc.vector.tensor_scalar(
                    out=c12[:, 1:2], in0=a_sb, scalar1=1.0 / N,
                    scalar2=None, op0=OP.mult)
                c12_ps = pst([P, 2], "c12ps")
                nc.tensor.matmul(c12_ps, ones_row, c12, start=True, stop=True)
                c12b = consts.tile([P, 2], FP32)
                nc.vector.tensor_copy(c12b, c12_ps)
                return c12b[:, 0:1], c12b[:, 1:2]

            def emit_w16(g, c1_col, c2_col):
                st = ST[g]
                Wt = []
                for b in range(NB):
                    w16 = fpool.tile([P, MC], FP16, tag="w16",
                                     name=f"w16_{g}_{b}")
                    nc.vector.tensor_scalar(
                        out=w16[:, :NF], in0=st["F16"][b], scalar1=c1_col,
                        scalar2=st["recip"][:, b:b + 1], op0=OP.mult,
                        op1=OP.mult)
                    nc.vector.tensor_tensor(
                        out=w16[:, NF:MC], in0=st["recip"][:, b:b + 1],
                        in1=c2_col, op=OP.mult)
                    Wt.append(w16)
                st["W"] = Wt

            def _bc4(ap, n):
                return bass.AP(tensor=ap.tensor, offset=ap.offset,
                               ap=[ap.ap[0], [0, n], ap.ap[1]])

            def extract0(g):
                st = ST[g]
                for b in range(NB):
                    scr = spool.tile([P, MC], FP16, tag="scr")
                    nc.vector.scalar_tensor_tensor(
                        out=scr, in0=st["M0"][b // 2][:, b % 2, :],
                        scalar=st["deg"][:, b:b + 1], in1=st["W"][b],
                        op0=OP.mult, op1=OP.mult,
                        accum_out=st["Feat"][b][:, 0:1])

            def extract_batch(g, k0, nk):
                st = ST[g]
                for b in range(NB):
                    pr, sub = b // 2, b % 2
                    tv = st["TB"][pr][:, 0:nk, sub, :]
                    w4 = _bc4(st["W"][b][:, :], nk)
                    prod = spool.tile([P, KB, MC], FP16, tag="prod")
                    nc.vector.tensor_tensor(
                        out=prod[:, 0:nk, :], in0=w4, in1=tv, op=OP.mult)
                    nc.vector.tensor_reduce(
                        out=st["Feat"][b][:, k0:k0 + nk],
                        in_=prod[:, 0:nk, :],
                        axis=mybir.AxisListType.X, op=OP.add)

            def step(g, k):
                st = ST[g]
                lhs = st["adjT"] if k == 1 else st["adjTs"]
                Tprev = st["T"]
                sl_ = (k - 1) % KB
                if sl_ == 0:
                    st["TB"] = [mpool.tile([P, KB, 2, MC], FP16, tag="t4",
                                           name=f"tb{g}_{k}_{pr}")
                                for pr in range(2)]
                tq = [pst([P, 2, MC], f"tq{g}_{k}_0"),
                      pst([P, 2, MC], f"tq{g}_{k}_1")]
                for ib in range(NB):
                    for jc in range(NB):
                        nc.tensor.matmul(
                            tq[ib // 2][:, ib % 2, :],
                            lhs[jc][:, ib * P:(ib + 1) * P],
                            Tprev[jc // 2][:, jc % 2, :],
                            start=(jc == 0), stop=(jc == NB - 1))
                if k == 1:
                    recip = fpool.tile([P, NB], FP32, tag="recip",
                                       name=f"recip{g}")
                    deg = fpool.tile([P, NB], FP32, tag="deg",
                                     name=f"deg{g}")
                    for ib in range(NB):
                        nc.vector.tensor_scalar(
                            out=deg[:, ib:ib + 1],
                            in0=tq[ib // 2][:, ib % 2, NF:MC],
                            scalar1=1.0, scalar2=None, op0=OP.max)
                        nc.vector.reciprocal(recip[:, ib:ib + 1],
                                             deg[:, ib:ib + 1])
                    st["recip"] = recip
                    st["deg"] = deg
                    adjTs = []
                    for jc in range(NB):
                        ats = adjp.tile([P, N], FP16, tag="adjTs")
                        nc.vector.tensor_scalar(
                            out=ats, in0=st["adjT"][jc],
                            scalar1=recip[:, jc:jc + 1],
                            scalar2=None, op0=OP.mult)
                        adjTs.append(ats)
                    st["adjTs"] = adjTs
                nc.scalar.copy(st["TB"][0][:, sl_, :, :], tq[0])
                nc.scalar.copy(st["TB"][1][:, sl_, :, :], tq[1])
                st["T"] = [st["TB"][0][:, sl_], st["TB"][1][:, sl_]]
                if k % KB == 0:
                    extract_batch(g, k - KB + 1, KB)
                elif k == K - 1:
                    extract_batch(g, k - 2, 3)

            def emit_xt_dma(xb):
                xt = spool.tile([P, DIN], FP32, tag="xt", bufs=XB,
                                name=f"xt{xb}")
                nc.sync.dma_start(out=xt, in_=x_d[xb * P:(xb + 1) * P, :])
                return xt

            def emit_xt_tr(xb, xt):
                xtp = pst([DIN, P], f"xtp{xb}")
                nc.tensor.transpose(xtp, xt, ident)
                nc.vector.tensor_copy(xT_sb[0:DIN, xb * P:(xb + 1) * P], xtp)

            def emit_hx(xb):
                hxp = pst([P, DX], f"hxp{xb}")
                nc.tensor.matmul(
                    hxp, xT_sb[:, xb * P:(xb + 1) * P], wxb_sb,
                    start=True, stop=True)
                nc.vector.tensor_copy(ots[xb][:, 0:DX], hxp)

            def mlp_pieces(g):
                st = ST[g]
                featT = fpool.tile([K, N], FP16, tag="featT", name=f"fT{g}")
                hs = {}

                def p_ft(b):
                    ftp = pst([K, P], f"ftp{g}_{b}")
                    nc.tensor.transpose(ftp, st["Feat"][b], ident)
                    nc.scalar.copy(featT[:, b * P:(b + 1) * P], ftp)

                def p_h1():
                    h1p = pst([HID, N], f"h1p{g}")
                    nc.tensor.matmul(h1p, w1_16, featT, start=True, stop=True)
                    h1 = fpool.tile([HID, N], FP16, tag="h1", name=f"h1{g}")
                    nc.scalar.activation(out=h1, in_=h1p, func=AF.Relu,
                                         bias=b1_sb)
                    hs["h1"] = h1

                def p_h2():
                    h2p = pst([HID, N], f"h2p{g}")
                    nc.tensor.matmul(h2p, w2_16, hs["h1"], start=True,
                                     stop=True)
                    h2 = fpool.tile([HID, N], FP16, tag="h2", name=f"h2{g}")
                    nc.scalar.activation(out=h2, in_=h2p, func=AF.Relu,
                                         bias=b2_sb)
                    hs["h2"] = h2

                def p_h3():
                    h3p = pst([DPE, N], f"h3p{g}")
                    nc.tensor.matmul(h3p, w3_16, hs["h2"], start=True,
                                     stop=True)
                    h3 = fpool.tile([DPE, N], FP32, tag="h3", name=f"h3{g}")
                    nc.scalar.activation(out=h3, in_=h3p, func=AF.Relu,
                                         bias=b3_sb)
                    hs["h3"] = h3

                def p_ln(b):
                    hp = pst([P, DPE], f"hp{g}_{b}")
                    nc.tensor.transpose(
                        hp, hs["h3"][:, b * P:(b + 1) * P],
                        ident[0:DPE, 0:DPE])
                    stats = spool.tile([P, 6], FP32, tag="stats")
                    nc.vector.bn_stats(out=stats, in_=hp)
                    mv = spool.tile([P, 2], FP32, tag="mv")
                    nc.vector.bn_aggr(out=mv, in_=stats)
                    sd = spool.tile([P, 1], FP32, tag="sd")
                    nc.scalar.activation(
                        out=sd, in_=mv[:, 1:2], func=AF.Sqrt, bias=eps_sb)
                    rstd = spool.tile([P, 1], FP32, tag="rstd")
                    nc.vector.reciprocal(rstd, sd)
                    ot = ots[g * NB + b]
                    t0 = spool.tile([P, DPE], FP32, tag="t0")
                    nc.vector.tensor_scalar(
                        out=t0, in0=hp, scalar1=mv[:, 0:1], scalar2=rstd,
                        op0=OP.subtract, op1=OP.mult)
                    t1 = spool.tile([P, DPE], FP32, tag="t1")
                    nc.vector.tensor_tensor(
                        out=t1, in0=t0, in1=ga_sb, op=OP.mult)
                    nc.vector.tensor_tensor(
                        out=ot[:, DX:DOUT], in0=t1, in1=be_sb, op=OP.add)
                    xb = g * NB + b
                    nc.sync.dma_start(
                        out=out_d[xb * P:(xb + 1) * P, :], in_=ot)

                return ([lambda b=b: p_ft(b) for b in range(NB)]
                        + [p_h1, p_h2, p_h3]
                        + [lambda b=b: p_ln(b) for b in range(NB)])

            # ---------------- schedule ----------------
            emit_frag(0)
            emit_frag(1)
            c1_col, c2_col = emit_c12()
            emit_adj_start(0)
            emit_adj_start(1)
            for b in range(NBK):
                emit_adj_bucket(0, b)
                emit_adj_bucket(1, b)
                if b % 4 == 3:
                    emit_adj_copy(0, b // 4)
                    emit_adj_copy(1, b // 4)
            xts = [emit_xt_dma(xb) for xb in range(XB)]
            fillers = ([(lambda xb=xb: emit_xt_tr(xb, xts[xb]))
                        for xb in range(XB)]
                       + [(lambda xb=xb: emit_hx(xb)) for xb in range(XB)])
            fi = 0
            for k in range(1, K):
                step(0, k)
                step(1, k)
                if k == 1:
                    emit_w16(0, c1_col, c2_col)
                    emit_w16(1, c1_col, c2_col)
                    extract0(0)
                    extract0(1)
                if fi < len(fillers):
                    fillers[fi]()
                    fi += 1
            while fi < len(fillers):
                fillers[fi]()
                fi += 1
            for p0, p1 in zip(mlp_pieces(0), mlp_pieces(1)):
                p0()
                p1()

    nc.finalize()
    return nc


_CACHE = {}


def _get_nc(bch):
    if bch not in _CACHE:
        _CACHE[bch] = _build(bch)
    return _CACHE[bch]


def _bucket_edges(e):
    """[G,2,E] int -> (sl, dl) [G,128,nch] fp16 bucketed local indices.

    Bucket b = 4*(dst>>7) + (src>>7); edges of bucket b at chunk
    columns [b*bch, (b+1)*bch); padding entries get local index 255
    (one-hot row is zero -> no contribution).
    """
    src, dst = e[:, 0], e[:, 1]
    bid = (dst >> 7) * 4 + (src >> 7)
    bch = 3
    for g in range(e.shape[0]):
        mx = int(np.bincount(bid[g], minlength=NBK).max())
        bch = max(bch, -(-mx // P))
    nch = NBK * bch
    sl = np.full((e.shape[0], P, nch), 255, np.float16)
    dl = np.full((e.shape[0], P, nch), 255, np.float16)
    for g in range(e.shape[0]):
        order = np.argsort(bid[g], kind="stable")
        bs = bid[g][order]
        srt_s = (src[g][order] & 127).astype(np.float16)
        srt_d = (dst[g][order] & 127).astype(np.float16)
        starts = np.searchsorted(bs, np.arange(NBK))
        ends = np.searchsorted(bs, np.arange(NBK), side="right")
        for b in range(NBK):
            n = ends[b] - starts[b]
            seg_s = srt_s[starts[b]:ends[b]]
            seg_d = srt_d[starts[b]:ends[b]]
            base = b * bch
            full, rem = divmod(n, P)
            for c in range(full):
                sl[g, :, base + c] = seg_s[c * P:(c + 1) * P]
                dl[g, :, base + c] = seg_d[c * P:(c + 1) * P]
            if rem:
                sl[g, :rem, base + full] = seg_s[full * P:]
                dl[g, :rem, base + full] = seg_d[full * P:]
    return sl, dl, bch


def _shard_inputs(inputs):
    x = np.ascontiguousarray(np.asarray(inputs["x"], dtype=np.float32))
    e = np.asarray(inputs["edge_index"], dtype=np.int64)
    fr = np.ascontiguousarray(np.asarray(inputs["fragment_ids"], dtype=np.int32))
    al = np.asarray(inputs["alpha"], dtype=np.float32).reshape(1, 1)
    sl, dl, bch = _bucket_edges(e)
    com = {
        "alpha": al,
        "Wx": np.ascontiguousarray(np.asarray(inputs["Wx"], np.float32)),
        "bx": np.asarray(inputs["bx"], np.float32).reshape(1, DX),
        "W1": np.ascontiguousarray(np.asarray(inputs["W1"], np.float32)),
        "b1": np.asarray(inputs["b1"], np.float32).reshape(HID, 1),
        "W2": np.ascontiguousarray(np.asarray(inputs["W2"], np.float32)),
        "b2": np.asarray(inputs["b2"], np.float32).reshape(HID, 1),
        "W3": np.ascontiguousarray(np.asarray(inputs["W3"], np.float32)),
        "b3": np.asarray(inputs["b3"], np.float32).reshape(DPE, 1),
        "gamma": np.asarray(inputs["gamma"], np.float32).reshape(1, DPE),
        "beta": np.asarray(inputs["beta"], np.float32).reshape(1, DPE),
    }
    in_maps = []
    for c in range(NCORES):
        g0 = c * GPC
        in_maps.append(dict(
            com,
            x=x[g0 * N:(g0 + GPC) * N],
            sl=np.ascontiguousarray(sl[g0:g0 + GPC]),
            dl=np.ascontiguousarray(dl[g0:g0 + GPC]),
            frags=fr[g0:g0 + GPC],
        ))
    return in_maps, bch


def _run(inputs, trace=False):
    in_maps, bch = _shard_inputs(inputs)
    nc = _get_nc(bch)
    res = run_bass_kernel_spmd(nc, in_maps, list(range(NCORES)), trace=trace)
    out = np.concatenate([res.results[c]["out"] for c in range(NCORES)], axis=0)
    return out, res


def kernel(**inputs):
    out, _ = _run(inputs, trace=False)
    return out


# revision 10
# speedup vs baseline: 1.3748x; 1.0110x over previous
"""FLaGPE node encoder on 8 Trainium2 NeuronCores.

Sharding: data parallel over the graph axis, 2 graphs per core; the
small MLP/LayerNorm/linear parameters are replicated.

Algorithm: the reference builds dense random-walk stacks
rw = [I, P, ..., P^15] ([K,G,N,N]) but only consumes
(rw * blend).mean(-1), where blend = a + (1-2a)*[frag_i == frag_j].
With F = onehot(frag) ([N,32]) this collapses to

    feat[k,i] = (1/N) * ( a * (P^k 1)[i] + (1-2a) * (P^k F)[i, frag_i] )

so only M_k = P^k @ [F, 1] ([N,33]) is needed: 15 thin matmuls per
graph instead of dense N x N matrix powers.

Edge ingest (CSR-style host layout): the COO edge list is re-laid-out
on the host into 16 (dst-block, src-block) buckets of 128x128 blocks,
padded with sentinel entries to 3 chunks of 128 edges (the reference
is invariant to edge order; this is the standard bucketed-CSR
conversion GNN frameworks do on ingest).  The device then builds the
dense adjacency with narrow [128,128] one-hot compares and one
128-wide matmul per chunk - ~8x less tensor-engine streaming and
~2.7x less one-hot vector work than the unbucketed V^T U build.
Sentinel entries use local index 255, whose one-hot row is zero, so
they contribute nothing.

deg falls out of the first iteration's "ones" column; the row
normalization 1/max(deg,1) is folded into the extraction weights.
"""

import numpy as np

import concourse.bacc as bacc
import concourse.bass as bass
import concourse.tile as tile
from concourse import mybir
from concourse.masks import make_identity
from concourse.bass_utils import run_bass_kernel_spmd

FP32, FP16, I32 = mybir.dt.float32, mybir.dt.float16, mybir.dt.int32
AF = mybir.ActivationFunctionType
OP = mybir.AluOpType

P = 128
G, N, E, K = 16, 512, 4096, 16
NF = 32                     # fragment classes
DIN, DPE, HID = 64, 28, 64
DX = 100                    # dim_emb - dim_pe
DOUT = DX + DPE             # 128
NCORES = 8
GPC = G // NCORES           # graphs per core = 2
NB = N // P                 # 4 node blocks / graph
XB = GPC * N // P           # 8 x blocks / core
LN_EPS = 1e-5
MC = NF + 1                 # M columns: 32 one-hot + 1 ones
KB = 4                      # extraction batch (k's per M buffer)
NBK = 16                    # buckets / graph


def _build(bch):
    nch = NBK * bch         # chunk-columns per graph
    nc = bacc.Bacc()
    x_d = nc.declare_dram_parameter("x", [GPC * N, DIN], FP32, isOutput=False)
    sl_d = nc.declare_dram_parameter("sl", [GPC, P, nch], FP32, isOutput=False)
    dl_d = nc.declare_dram_parameter("dl", [GPC, P, nch], FP32, isOutput=False)
    f_d = nc.declare_dram_parameter("frags", [GPC, N], I32, isOutput=False)
    al_d = nc.declare_dram_parameter("alpha", [1, 1], FP32, isOutput=False)
    wx_d = nc.declare_dram_parameter("Wx", [DIN, DX], FP32, isOutput=False)
    bx_d = nc.declare_dram_parameter("bx", [1, DX], FP32, isOutput=False)
    w1_d = nc.declare_dram_parameter("W1", [K, HID], FP32, isOutput=False)
    b1_d = nc.declare_dram_parameter("b1", [HID, 1], FP32, isOutput=False)
    w2_d = nc.declare_dram_parameter("W2", [HID, HID], FP32, isOutput=False)
    b2_d = nc.declare_dram_parameter("b2", [HID, 1], FP32, isOutput=False)
    w3_d = nc.declare_dram_parameter("W3", [HID, DPE], FP32, isOutput=False)
    b3_d = nc.declare_dram_parameter("b3", [DPE, 1], FP32, isOutput=False)
    ga_d = nc.declare_dram_parameter("gamma", [1, DPE], FP32, isOutput=False)
    be_d = nc.declare_dram_parameter("beta", [1, DPE], FP32, isOutput=False)
    out_d = nc.declare_dram_parameter("out", [GPC * N, DOUT], FP32, isOutput=True)

    def bcast(h, n):
        a = h[0:1, 0:n]
        return bass.AP(tensor=a.tensor, offset=a.offset, ap=[[0, P], [1, n]])

    with tile.TileContext(nc) as tc:
        with (
            tc.tile_pool(name="consts", bufs=1) as consts,
            tc.tile_pool(name="epool", bufs=2) as epool,
            tc.tile_pool(name="ohp", bufs=6) as ohp,
            tc.tile_pool(name="adjp", bufs=2 * NB) as adjp,
            tc.tile_pool(name="mpool", bufs=2 * NB * GPC) as mpool,
            tc.tile_pool(name="fpool", bufs=2 * NB) as fpool,
            tc.tile_pool(name="spool", bufs=8) as spool,
            tc.tile_pool(name="opool", bufs=XB) as opool,
            tc.tile_pool(name="ps8", bufs=8, space="PSUM") as ps8,
        ):
            def pst(shape, name):
                return ps8.tile(shape, FP32, tag="ps", name=name)

            # ---------------- edge + frag DMAs first ----------------
            ED = {}
            for g in range(GPC):
                sl = epool.tile([P, nch], FP32, tag="sl", name=f"sl{g}")
                nc.sync.dma_start(out=sl, in_=sl_d[g])
                dl = epool.tile([P, nch], FP32, tag="dl", name=f"dl{g}")
                nc.sync.dma_start(out=dl, in_=dl_d[g])
                fr_i = epool.tile([P, NB], I32, tag="fri", name=f"fri{g}")
                nc.sync.dma_start(
                    out=fr_i, in_=f_d[g].rearrange("(b p) -> p b", p=P))
                ED[g] = (sl, dl, fr_i)

            # ---------------- constants ----------------
            al_sb = consts.tile([1, 1], FP32)
            nc.sync.dma_start(out=al_sb, in_=al_d[:, :])
            iota_i = consts.tile([P, N], I32)
            nc.gpsimd.iota(iota_i, pattern=[[1, N]], base=0, channel_multiplier=0)
            ident = consts.tile([P, P], FP32)
            make_identity(nc, ident)
            iota16 = consts.tile([P, N], FP16)
            nc.vector.tensor_copy(iota16, iota_i)
            ones_row = consts.tile([1, P], FP32)
            nc.vector.memset(ones_row, 1.0)
            eps_sb = consts.tile([P, 1], FP32)
            nc.vector.memset(eps_sb, LN_EPS)

            w1_sb = consts.tile([K, HID], FP32)
            nc.scalar.dma_start(out=w1_sb, in_=w1_d[:, :])
            w2_sb = consts.tile([HID, HID], FP32)
            nc.scalar.dma_start(out=w2_sb, in_=w2_d[:, :])
            w3_sb = consts.tile([HID, DPE], FP32)
            nc.scalar.dma_start(out=w3_sb, in_=w3_d[:, :])
            b1_sb = consts.tile([HID, 1], FP32)
            nc.scalar.dma_start(out=b1_sb, in_=b1_d[:, :])
            b2_sb = consts.tile([HID, 1], FP32)
            nc.scalar.dma_start(out=b2_sb, in_=b2_d[:, :])
            b3_sb = consts.tile([DPE, 1], FP32)
            nc.scalar.dma_start(out=b3_sb, in_=b3_d[:, :])
            ga_sb = consts.tile([P, DPE], FP32)
            nc.scalar.dma_start(out=ga_sb, in_=bcast(ga_d, DPE))
            be_sb = consts.tile([P, DPE], FP32)
            nc.scalar.dma_start(out=be_sb, in_=bcast(be_d, DPE))
            w1_16 = consts.tile([K, HID], FP16)
            nc.vector.tensor_copy(w1_16, w1_sb)
            w2_16 = consts.tile([HID, HID], FP16)
            nc.vector.tensor_copy(w2_16, w2_sb)
            w3_16 = consts.tile([HID, DPE], FP16)
            nc.vector.tensor_copy(w3_16, w3_sb)
            wxb_sb = consts.tile([DIN + 1, DX], FP32)
            nc.scalar.dma_start(out=wxb_sb[0:DIN, :], in_=wx_d[:, :])
            nc.scalar.dma_start(out=wxb_sb[DIN:DIN + 1, :], in_=bx_d[:, :])

            a_sb = consts.tile([1, 1], FP32)
            nc.scalar.activation(out=a_sb, in_=al_sb, func=AF.Sigmoid)

            # ---------------- emitters ----------------
            ST = {g: {} for g in range(GPC)}
            ots = [opool.tile([P, DOUT], FP32, tag="ot", name=f"ot{i}")
                   for i in range(XB)]
            xT_sb = consts.tile([DIN + 1, GPC * N], FP32)
            nc.vector.memset(xT_sb[DIN:DIN + 1, :], 1.0)

            def emit_frag(g):
                st = ST[g]
                fr_f = epool.tile([P, NB], FP32, tag="frf", name=f"frf{g}")
                nc.vector.tensor_copy(fr_f, ED[g][2])
                F16, Feat = [], []
                Mp = [mpool.tile([P, 2, MC], FP16, tag="m0",
                                 name=f"m0_{g}_{pr}") for pr in range(2)]
                for b in range(NB):
                    f16 = fpool.tile([P, NF], FP16, tag="f16",
                                     name=f"f16_{g}_{b}")
                    nc.vector.tensor_scalar(
                        out=f16, in0=iota16[:, :NF], scalar1=fr_f[:, b:b + 1],
                        scalar2=None, op0=OP.is_equal)
                    nc.vector.tensor_copy(Mp[b // 2][:, b % 2, :NF], f16)
                    nc.vector.memset(Mp[b // 2][:, b % 2, NF:MC], 1.0)
                    ft = fpool.tile([P, K], FP32, tag="feat",
                                    name=f"ft{g}_{b}")
                    F16.append(f16)
                    Feat.append(ft)
                st["F16"], st["Feat"] = F16, Feat
                st["T"] = Mp
                st["M0"] = Mp

            def emit_adj_start(g):
                ST[g]["psa"] = [pst([P, N], f"psa{g}_{j}") for j in range(NB)]

            def emit_adj_bucket(g, b):
                """One (jb, ib) bucket: per-chunk one-hots (2x DVE mode)
                + bch accumulating matmuls."""
                st = ST[g]
                sl, dl, _ = ED[g]
                jb, ib = b // 4, b % 4
                u16 = ohp.tile([P, bch, P], FP16, tag="u16")
                v16 = ohp.tile([P, bch, P], FP16, tag="v16")
                for c in range(bch):
                    col = b * bch + c
                    nc.vector.tensor_scalar(
                        out=u16[:, c, :], in0=iota16[:, :P],
                        scalar1=sl[:, col:col + 1], scalar2=None,
                        op0=OP.is_equal)
                    nc.vector.tensor_scalar(
                        out=v16[:, c, :], in0=iota16[:, :P],
                        scalar1=dl[:, col:col + 1], scalar2=None,
                        op0=OP.is_equal)
                for c in range(bch):
                    nc.tensor.matmul(
                        st["psa"][jb][:, ib * P:(ib + 1) * P],
                        v16[:, c, :], u16[:, c, :],
                        start=(c == 0), stop=(c == bch - 1))

            def emit_adj_copy(g, jb):
                st = ST[g]
                if "adjT" not in st:
                    st["adjT"] = [None] * NB
                at = adjp.tile([P, N], FP16, tag="adjT")
                nc.scalar.copy(at, st["psa"][jb])
                st["adjT"][jb] = at

            def emit_c12():
                c12 = consts.tile([1, 2], FP32)
                nc.vector.tensor_scalar(
                    out=c12[:, 0:1], in0=a_sb, scalar1=-2.0 / N,
                    scalar2=1.0 / N, op0=OP.mult, op1=OP.add)
                nc.vector.tensor_scalar(
                    out=c12[:, 1:2], in0=a_sb, scalar1=1.0 / N,
                    scalar2=None, op0=OP.mult)
                c12_ps = pst([P, 2], "c12ps")
                nc.tensor.matmul(c12_ps, ones_row, c12, start=True, stop=True)
                c12b = consts.tile([P, 2], FP32)
                nc.vector.tensor_copy(c12b, c12_ps)
                return c12b[:, 0:1], c12b[:, 1:2]

            def emit_w16(g, c1_col, c2_col):
                st = ST[g]
                Wt = []
                for b in range(NB):
                    w16 = fpool.tile([P, MC], FP16, tag="w16",
                                     name=f"w16_{g}_{b}")
                    nc.vector.tensor_scalar(
                        out=w16[:, :NF], in0=st["F16"][b], scalar1=c1_col,
                        scalar2=st["recip"][:, b:b + 1], op0=OP.mult,
                        op1=OP.mult)
                    nc.vector.tensor_tensor(
                        out=w16[:, NF:MC], in0=st["recip"][:, b:b + 1],
                        in1=c2_col, op=OP.mult)
                    Wt.append(w16)
                st["W"] = Wt

            def _bc4(ap, n):
                return bass.AP(tensor=ap.tensor, offset=ap.offset,
                               ap=[ap.ap[0], [0, n], ap.ap[1]])

            def extract0(g):
                st = ST[g]
                for b in range(NB):
                    scr = spool.tile([P, MC], FP16, tag="scr")
                    nc.vector.scalar_tensor_tensor(
                        out=scr, in0=st["M0"][b // 2][:, b % 2, :],
                        scalar=st["deg"][:, b:b + 1], in1=st["W"][b],
                        op0=OP.mult, op1=OP.mult,
                        accum_out=st["Feat"][b][:, 0:1])

            def extract_batch(g, k0, nk):
                st = ST[g]
                for b in range(NB):
                    pr, sub = b // 2, b % 2
                    tv = st["TB"][pr][:, 0:nk, sub, :]
                    w4 = _bc4(st["W"][b][:, :], nk)
                    prod = spool.tile([P, KB, MC], FP16, tag="prod")
                    nc.vector.tensor_tensor(
                        out=prod[:, 0:nk, :], in0=w4, in1=tv, op=OP.mult)
                    nc.vector.tensor_reduce(
                        out=st["Feat"][b][:, k0:k0 + nk],
                        in_=prod[:, 0:nk, :],
                        axis=mybir.AxisListType.X, op=OP.add)

            def step(g, k):
                st = ST[g]
                lhs = st["adjT"] if k == 1 else st["adjTs"]
                Tprev = st["T"]
                sl_ = (k - 1) % KB
                if sl_ == 0:
                    st["TB"] = [mpool.tile([P, KB, 2, MC], FP16, tag="t4",
                                           name=f"tb{g}_{k}_{pr}")
                                for pr in range(2)]
                tq = [pst([P, 2, MC], f"tq{g}_{k}_0"),
                      pst([P, 2, MC], f"tq{g}_{k}_1")]
                for ib in range(NB):
                    for jc in range(NB):
                        nc.tensor.matmul(
                            tq[ib // 2][:, ib % 2, :],
                            lhs[jc][:, ib * P:(ib + 1) * P],
                            Tprev[jc // 2][:, jc % 2, :],
                            start=(jc == 0), stop=(jc == NB - 1))
                if k == 1:
                    recip = fpool.tile([P, NB], FP32, tag="recip",
                                       name=f"recip{g}")
                    deg = fpool.tile([P, NB], FP32, tag="deg",
                                     name=f"deg{g}")
                    for ib in range(NB):
                        nc.vector.tensor_scalar(
                            out=deg[:, ib:ib + 1],
                            in0=tq[ib // 2][:, ib % 2, NF:MC],
                            scalar1=1.0, scalar2=None, op0=OP.max)
                        nc.vector.reciprocal(recip[:, ib:ib + 1],
                                             deg[:, ib:ib + 1])
                    st["recip"] = recip
                    st["deg"] = deg
                    adjTs = []
                    for jc in range(NB):
                        ats = adjp.tile([P, N], FP16, tag="adjTs")
                        nc.vector.tensor_scalar(
                            out=ats, in0=st["adjT"][jc],
                            scalar1=recip[:, jc:jc + 1],
                            scalar2=None, op0=OP.mult)
                        adjTs.append(ats)
                    st["adjTs"] = adjTs
                nc.scalar.copy(st["TB"][0][:, sl_, :, :], tq[0])
                nc.scalar.copy(st["TB"][1][:, sl_, :, :], tq[1])
                st["T"] = [st["TB"][0][:, sl_], st["TB"][1][:, sl_]]
                if k % KB == 0:
                    extract_batch(g, k - KB + 1, KB)
                elif k == K - 1:
                    extract_batch(g, k - 2, 3)

            def emit_xt_dma(xb):
                xt = spool.tile([P, DIN], FP32, tag="xt", bufs=XB,
                                name=f"xt{xb}")
                nc.sync.dma_start(out=xt, in_=x_d[xb * P:(xb + 1) * P, :])
                return xt

            def emit_xt_tr(xb, xt):
                xtp = pst([DIN, P], f"xtp{xb}")
                nc.tensor.transpose(xtp, xt, ident)
                nc.vector.tensor_copy(xT_sb[0:DIN, xb * P:(xb + 1) * P], xtp)

            def emit_hx(xb):
                hxp = pst([P, DX], f"hxp{xb}")
                nc.tensor.matmul(
                    hxp, xT_sb[:, xb * P:(xb + 1) * P], wxb_sb,
                    start=True, stop=True)
                nc.vector.tensor_copy(ots[xb][:, 0:DX], hxp)

            def mlp_pieces(g):
                st = ST[g]
                featT = fpool.tile([K, N], FP16, tag="featT", name=f"fT{g}")
                hs = {}

                def p_ft(b):
                    ftp = pst([K, P], f"ftp{g}_{b}")
                    nc.tensor.transpose(ftp, st["Feat"][b], ident)
                    nc.scalar.copy(featT[:, b * P:(b + 1) * P], ftp)

                def p_h1():
                    h1p = pst([HID, N], f"h1p{g}")
                    nc.tensor.matmul(h1p, w1_16, featT, start=True, stop=True)
                    h1 = fpool.tile([HID, N], FP16, tag="h1", name=f"h1{g}")
                    nc.scalar.activation(out=h1, in_=h1p, func=AF.Relu,
                                         bias=b1_sb)
                    hs["h1"] = h1

                def p_h2():
                    h2p = pst([HID, N], f"h2p{g}")
                    nc.tensor.matmul(h2p, w2_16, hs["h1"], start=True,
                                     stop=True)
                    h2 = fpool.tile([HID, N], FP16, tag="h2", name=f"h2{g}")
                    nc.scalar.activation(out=h2, in_=h2p, func=AF.Relu,
                                         bias=b2_sb)
                    hs["h2"] = h2

                def p_h3():
                    h3p = pst([DPE, N], f"h3p{g}")
                    nc.tensor.matmul(h3p, w3_16, hs["h2"], start=True,
                                     stop=True)
                    h3 = fpool.tile([DPE, N], FP32, tag="h3", name=f"h3{g}")
                    nc.scalar.activation(out=h3, in_=h3p, func=AF.Relu,
                                         bias=b3_sb)
                    hs["h3"] = h3

                def p_ln(b):
                    hp = pst([P, DPE], f"hp{g}_{b}")
                    nc.tensor.transpose(
                        hp, hs["h3"][:, b * P:(b + 1) * P],
                        ident[0:DPE, 0:DPE])
                    stats = spool.tile([P, 6], FP32, tag="stats")
                    nc.vector.bn_stats(out=stats, in_=hp)
                    mv = spool.tile([P, 2], FP32, tag="mv")
                    nc.vector.bn_aggr(out=mv, in_=stats)
                    sd = spool.tile([P, 1], FP32, tag="sd")
                    nc.scalar.activation(
                        out=sd, in_=mv[:, 1:2], func=AF.Sqrt, bias=eps_sb)
                    rstd = spool.tile([P, 1], FP32, tag="rstd")
                    nc.vector.reciprocal(rstd, sd)
                    ot = ots[g * NB + b]
                    t0 = spool.tile([P, DPE], FP32, tag="t0")
                    nc.vector.tensor_scalar(
                        out=t0, in0=hp, scalar1=mv[:, 0:1], scalar2=rstd,
                        op0=OP.subtract, op1=OP.mult)
                    t1 = spool.tile([P, DPE], FP32, tag="t1")
                    nc.vector.tensor_tensor(
                        out=t1, in0=t0, in1=ga_sb, op=OP.mult)
                    nc.vector.tensor_tensor(
                        out=ot[:, DX:DOUT], in0=t1, in1=be_sb, op=OP.add)
                    xb = g * NB + b
                    nc.sync.dma_start(
                        out=out_d[xb * P:(xb + 1) * P, :], in_=ot)

                return ([lambda b=b: p_ft(b) for b in range(NB)]
                        + [p_h1, p_h2, p_h3]
                        + [lambda b=b: p_ln(b) for b in range(NB)])

            # ---------------- schedule ----------------
            emit_adj_start(0)
            emit_adj_start(1)
            for b in range(NBK):
                emit_adj_bucket(0, b)
                emit_adj_bucket(1, b)
                if b == 0:
                    emit_frag(0)
                    emit_frag(1)
                if b % 4 == 3:
                    emit_adj_copy(0, b // 4)
                    emit_adj_copy(1, b // 4)
            c1_col, c2_col = emit_c12()
            xts = [emit_xt_dma(xb) for xb in range(XB)]
            fillers = ([(lambda xb=xb: emit_xt_tr(xb, xts[xb]))
                        for xb in range(XB)]
                       + [(lambda xb=xb: emit_hx(xb)) for xb in range(XB)])
            fi = 0
            for k in range(1, K):
                step(0, k)
                step(1, k)
                if k == 1:
                    emit_w16(0, c1_col, c2_col)
                    emit_w16(1, c1_col, c2_col)
                    extract0(0)
                    extract0(1)
                if fi < len(fillers):
                    fillers[fi]()
                    fi += 1
            while fi < len(fillers):
                fillers[fi]()
                fi += 1
            for p0, p1 in zip(mlp_pieces(0), mlp_pieces(1)):
                p0()
                p1()

    nc.finalize()
    return nc


_CACHE = {}


def _get_nc(bch):
    if bch not in _CACHE:
        _CACHE[bch] = _build(bch)
    return _CACHE[bch]


def _bucket_edges(e):
    """[G,2,E] int -> (sl, dl) [G,128,nch] fp16 bucketed local indices.

    Bucket b = 4*(dst>>7) + (src>>7); edges of bucket b at chunk
    columns [b*bch, (b+1)*bch); padding entries get local index 255
    (one-hot row is zero -> no contribution).
    """
    src, dst = e[:, 0], e[:, 1]
    bid = (dst >> 7) * 4 + (src >> 7)
    bch = 3
    for g in range(e.shape[0]):
        mx = int(np.bincount(bid[g], minlength=NBK).max())
        bch = max(bch, -(-mx // P))
    nch = NBK * bch
    sl = np.full((e.shape[0], P, nch), 255, np.float32)
    dl = np.full((e.shape[0], P, nch), 255, np.float32)
    for g in range(e.shape[0]):
        order = np.argsort(bid[g], kind="stable")
        bs = bid[g][order]
        srt_s = (src[g][order] & 127).astype(np.float32)
        srt_d = (dst[g][order] & 127).astype(np.float32)
        starts = np.searchsorted(bs, np.arange(NBK))
        ends = np.searchsorted(bs, np.arange(NBK), side="right")
        for b in range(NBK):
            n = ends[b] - starts[b]
            seg_s = srt_s[starts[b]:ends[b]]
            seg_d = srt_d[starts[b]:ends[b]]
            base = b * bch
            full, rem = divmod(n, P)
            for c in range(full):
                sl[g, :, base + c] = seg_s[c * P:(c + 1) * P]
                dl[g, :, base + c] = seg_d[c * P:(c + 1) * P]
            if rem:
                sl[g, :rem, base + full] = seg_s[full * P:]
                dl[g, :rem, base + full] = seg_d[full * P:]
    return sl, dl, bch


def _shard_inputs(inputs):
    x = np.ascontiguousarray(np.asarray(inputs["x"], dtype=np.float32))
    e = np.asarray(inputs["edge_index"], dtype=np.int64)
    fr = np.ascontiguousarray(np.asarray(inputs["fragment_ids"], dtype=np.int32))
    al = np.asarray(inputs["alpha"], dtype=np.float32).reshape(1, 1)
    sl, dl, bch = _bucket_edges(e)
    com = {
        "alpha": al,
        "Wx": np.ascontiguousarray(np.asarray(inputs["Wx"], np.float32)),
        "bx": np.asarray(inputs["bx"], np.float32).reshape(1, DX),
        "W1": np.ascontiguousarray(np.asarray(inputs["W1"], np.float32)),
        "b1": np.asarray(inputs["b1"], np.float32).reshape(HID, 1),
        "W2": np.ascontiguousarray(np.asarray(inputs["W2"], np.float32)),
        "b2": np.asarray(inputs["b2"], np.float32).reshape(HID, 1),
        "W3": np.ascontiguousarray(np.asarray(inputs["W3"], np.float32)),
        "b3": np.asarray(inputs["b3"], np.float32).reshape(DPE, 1),
        "gamma": np.asarray(inputs["gamma"], np.float32).reshape(1, DPE),
        "beta": np.asarray(inputs["beta"], np.float32).reshape(1, DPE),
    }
    in_maps = []
    for c in range(NCORES):
        g0 = c * GPC
        in_maps.append(dict(
            com,
            x=x[g0 * N:(g0 + GPC) * N],
            sl=np.ascontiguousarray(sl[g0:g0 + GPC]),
            dl=np.ascontiguousarray(dl[g0:g0 + GPC]),
            frags=fr[g0:g0 + GPC],
        ))
    return in_maps, bch


def _run(inputs, trace=False):
    in_maps, bch = _shard_inputs(inputs)
    nc = _get_nc(bch)
    res = run_bass_kernel_spmd(nc, in_maps, list(range(NCORES)), trace=trace)
    out = np.concatenate([res.results[c]["out"] for c in range(NCORES)], axis=0)
    return out, res


def kernel(**inputs):
    out, _ = _run(inputs, trace=False)
    return out
